# revision 8
# baseline (speedup 1.0000x reference)
"""ASpTLinear Trainium2 kernel: out = x @ W^T + bias over 8 NeuronCores.

Sharding: tokens (B*S=8192) split 8 ways; each core computes
out^T_shard[4096, 1024] = W @ x_shard^T + bias as a float32r (fp24 =
sign/8exp/11-bit-mantissa, full PE rate) tensor-engine GEMM:

  - x shard lives SBUF-resident as [128, 32, 1024] (k-tile-major),
  - W streams through SBUF once per core in 32 column-blocks of
    [4096, 128], pre-permuted on the host to a fully-contiguous
    per-partition blocked layout (16 KB DMA lines),
  - each o-tile runs two 32-matmul PSUM accumulation groups (mt-major:
    consecutive matmuls hit the same PSUM bank),
  - ScalarE evicts PSUM -> SBUF adding the per-feature bias,
  - host pre-rounds inputs with RNE dropping 12 mantissa bits, which is
    bit-exact with the hardware's fp32r rounding, and pre-transposes so
    every DMA is a contiguous-line load.
"""
import numpy as np

P = 128
B, S, D_IN, D_OUT = 4, 2048, 4096, 4096
N_CORES = 8
M_TOT = B * S                 # 8192 tokens
M_SH = M_TOT // N_CORES       # 1024 tokens per core
KT = D_IN // P                # 32 contraction tiles
OT = D_OUT // P               # 32 output-feature tiles
MF = 512                      # matmul moving free dim (one fp32 PSUM bank)
MT = M_SH // MF               # 2 m-tiles per core

_CACHE = {}


def _round_fp32r(a: np.ndarray) -> np.ndarray:
    """Round-to-nearest-even dropping the 12 low mantissa bits (HW fp32r)."""
    b = a.reshape(-1).view(np.uint32).astype(np.uint64)
    add = ((b >> 12) & 1) + 0x7FF
    b = (b + add) & ~np.uint64(0xFFF)
    return b.astype(np.uint32).view(np.float32).reshape(a.shape)


def build_nc(rep: int = 1, wbufs: int = 3, psbufs: int = 2):
    """rep>1 wraps the whole body in a hardware loop (timing only)."""
    import concourse.bacc as bacc
    import concourse.mybir as mybir
    from concourse.tile import TileContext

    nc = bacc.Bacc(None, target_bir_lowering=False, debug=False)
    xT = nc.declare_dram_parameter("xT", [D_IN, M_SH], mybir.dt.float32r, isOutput=False)
    wB = nc.declare_dram_parameter("wB", [OT, P, KT * P], mybir.dt.float32r,
                                   isOutput=False)
    bias = nc.declare_dram_parameter("bias", [D_OUT], mybir.dt.float32, isOutput=False)
    outT = nc.declare_dram_parameter("outT", [D_OUT, M_SH], mybir.dt.float32,
                                     isOutput=True)

    with TileContext(nc) as tc:
        with tc.tile_pool(name="xpool", bufs=1) as xpool, \
             tc.tile_pool(name="wpool", bufs=wbufs) as wpool, \
             tc.tile_pool(name="bpool", bufs=1) as bpool, \
             tc.tile_pool(name="opool", bufs=4) as opool, \
             tc.tile_pool(name="warmp", bufs=1) as warmp, \
             tc.tile_pool(name="pspool", bufs=psbufs, space="PSUM") as pspool, \
             tc.tile_pool(name="wps", bufs=1, space="PSUM") as wps:

            def body():
                x_sb = xpool.tile([P, KT, M_SH], mybir.dt.float32r, tag="x",
                                  name="x_sb")
                bias_sb = bpool.tile([P, OT], mybir.dt.float32, tag="b",
                                     name="bias_sb")
                w_tiles = {}

                def wdma(ot):
                    w_sb = wpool.tile([P, KT, P], mybir.dt.float32r, tag="w",
                                      name="w_sb")
                    nc.sync.dma_start(
                        out=w_sb[:],
                        in_=wB[ot].rearrange("p (kt o) -> p kt o", o=P))
                    return w_sb

                warm = warmp.tile([P, MF], mybir.dt.float32r, tag="warm",
                                  name="warm")
                nc.sync.dma_start(out=warm[:], in_=xT[0:P, 0:MF])
                wpsum = wps.tile([P, MF], mybir.dt.float32, tag="wpsum",
                                 name="wpsum")
                # first two W blocks issue ahead of the x slab so the PE can
                # start as soon as the first m-half of x lands
                for ot in range(2):
                    w_tiles[ot] = wdma(ot)
                for mh in range(MT):
                    for k in range(KT):
                        nc.sync.dma_start(
                            out=x_sb[:, k, mh*MF:(mh+1)*MF],
                            in_=xT[k*P:(k+1)*P, mh*MF:(mh+1)*MF])
                nc.sync.dma_start(out=bias_sb[:],
                                  in_=bias.rearrange("(ot p) -> p ot", p=P))
                # dummy matmuls keep the PE busy while x streams in, so the
                # HAM clock-gate is already at full rate when real work starts
                # (128 x ~213ns bridges the ~27us until the x m0-half lands)
                for _ in range(128):
                    nc.tensor.matmul(wpsum[:], lhsT=warm[:, 0:P], rhs=warm[:],
                                     start=True, stop=True)

                for ot in range(OT):
                    w_sb = w_tiles.pop(ot) if ot in w_tiles else wdma(ot)
                    for mt in range(MT):
                        ps = pspool.tile([P, MF], mybir.dt.float32, tag="ps",
                                         name="ps")
                        for k in range(KT):
                            nc.tensor.matmul(
                                ps[:], lhsT=w_sb[:, k, :],
                                rhs=x_sb[:, k, mt*MF:(mt+1)*MF],
                                start=(k == 0), stop=(k == KT - 1))
                        ob = opool.tile([P, MF], mybir.dt.float32, tag="ob",
                                        name="ob")
                        nc.scalar.activation(
                            ob[:], ps[:],
                            mybir.ActivationFunctionType.Identity,
                            bias=bias_sb[:, ot:ot+1])
                        # out-DMA issued by ScalarE's HWDGE: no cross-engine
                        # wait, and it keeps the SP queue free for W/x loads
                        nc.scalar.dma_start(
                            out=outT[ot*P:(ot+1)*P, mt*MF:(mt+1)*MF], in_=ob[:])

            if rep == 1:
                body()
            else:
                with tc.For_i(0, rep, 1) as _i:
                    body()
    nc.compile()
    return nc


class _Runner:
    """Compile a Bass module into a jitted n-core PJRT callable.

    Input names in `replicated` are fed once (every core receives the same
    array) instead of concatenated per-core.
    """

    def __init__(self, nc, n_cores, replicated=()):
        import jax
        import concourse.mybir as mybir
        from concourse import bass2jax
        from jax.experimental.shard_map import shard_map
        from jax.sharding import Mesh, PartitionSpec, NamedSharding

        bass2jax.install_neuronx_cc_hook()
        self.jax = jax
        self.n_cores = n_cores
        self.replicated = set(replicated)
        partition_name = (
            nc.partition_id_tensor.name if nc.partition_id_tensor else None)
        in_names, out_names, out_avals, zero_outs = [], [], [], []
        for alloc in nc.m.functions[0].allocations:
            if not isinstance(alloc, mybir.MemoryLocationSet):
                continue
            name = alloc.memorylocations[0].name
            if alloc.kind == "ExternalInput":
                if name != partition_name:
                    in_names.append(name)
            elif alloc.kind == "ExternalOutput":
                out_names.append(name)
                shape = tuple(alloc.tensor_shape)
                dtype = mybir.dt.np(alloc.dtype)
                out_avals.append(jax.core.ShapedArray(shape, dtype))
                zero_outs.append(np.zeros(shape, dtype))
        self.in_names, self.out_names = in_names, out_names
        self.out_avals, self.zero_outs = out_avals, zero_outs

        all_in_names = in_names + out_names
        if partition_name is not None:
            all_in_names.append(partition_name)

        def _body(*args):
            operands = list(args)
            if partition_name is not None:
                operands.append(bass2jax.partition_id_tensor())
            return tuple(bass2jax._bass_exec_p.bind(
                *operands,
                out_avals=tuple(out_avals),
                in_names=tuple(all_in_names),
                out_names=tuple(out_names),
                lowering_input_output_aliases=(),
                sim_require_finite=False,
                sim_require_nnan=False,
                nc=nc,
            ))

        devices = jax.devices()[:n_cores]
        assert len(devices) == n_cores, f"need {n_cores} neuron cores"
        self.mesh = Mesh(np.asarray(devices), ("core",))
        in_specs = tuple(
            PartitionSpec() if n in self.replicated else PartitionSpec("core")
            for n in in_names) + (PartitionSpec("core"),) * len(out_names)
        self._fn = jax.jit(
            shard_map(_body, mesh=self.mesh,
                      in_specs=in_specs,
                      out_specs=(PartitionSpec("core"),) * len(out_names),
                      check_rep=False),
            keep_unused=True)
        self._sharding = NamedSharding(self.mesh, PartitionSpec("core"))
        self._repl_sharding = NamedSharding(self.mesh, PartitionSpec())

    def place_inputs(self, in_maps):
        import jax.numpy as jnp
        args = []
        for name in self.in_names:
            if name in self.replicated:
                args.append(self.jax.device_put(
                    np.asarray(in_maps[0][name]), self._repl_sharding))
            else:
                args.append(self.jax.device_put(np.concatenate(
                    [np.asarray(m[name]) for m in in_maps], axis=0),
                    self._sharding))
        for z in self.zero_outs:
            shape = (self.n_cores * z.shape[0], *z.shape[1:])
            args.append(self.jax.jit(
                lambda shape=shape, dt=z.dtype: jnp.zeros(shape, dt),
                out_shardings=self._sharding)())
        return args

    def run(self, dev_args):
        outs = self._fn(*dev_args)
        self.jax.block_until_ready(outs)
        return outs

    def results(self, outs):
        res = [{} for _ in range(self.n_cores)]
        for i, name in enumerate(self.out_names):
            a = np.asarray(outs[i]).reshape(
                self.n_cores, *self.out_avals[i].shape)
            for c in range(self.n_cores):
                res[c][name] = a[c]
        return res


def _get_runner():
    if "runner" not in _CACHE:
        _CACHE["runner"] = _Runner(build_nc(), N_CORES,
                                   replicated=("wB", "bias"))
    return _CACHE["runner"]


def _prep_inputs(x, weight, bias):
    x2 = np.ascontiguousarray(x, dtype=np.float32).reshape(M_TOT, D_IN)
    xr = _round_fp32r(x2)
    wr = _round_fp32r(np.ascontiguousarray(weight, dtype=np.float32))
    # blocked layout: wB[ot, p, kt*P + o] = W[ot*P+o, kt*P+p]
    wB = np.ascontiguousarray(
        wr.T.reshape(KT, P, OT, P).transpose(2, 1, 0, 3).reshape(OT, P, KT * P))
    b = np.ascontiguousarray(bias, dtype=np.float32)
    return [{"xT": np.ascontiguousarray(xr[c*M_SH:(c+1)*M_SH, :].T),
             "wB": wB, "bias": b} for c in range(N_CORES)]


def kernel(x, weight, bias):
    in_maps = _prep_inputs(x, weight, bias)
    for attempt in range(2):
        try:
            r = _get_runner()
            dev_args = r.place_inputs(in_maps)
            res = r.results(r.run(dev_args))
            break
        except Exception:
            _CACHE.pop("runner", None)
            if attempt == 1:
                raise
            import time
            time.sleep(10)
    outT = np.concatenate([res[c]["outT"] for c in range(N_CORES)], axis=1)
    return np.ascontiguousarray(outT.T).reshape(B, S, D_OUT)
